# revision 1
# baseline (speedup 1.0000x reference)
"""Trainium2 Bass kernel for nn_Decoder (Bahdanau attention + 1-step LSTM + vocab fc).

Sharding: batch-parallel attention/LSTM across 8 NeuronCores (16 rows each);
the 32000-wide fc layer is vocab-parallel (4000 per core) fed by an on-device
AllGather of the LSTM hidden state. Embedding rows are gathered on the host
(indices are host-visible), so the emb table never touches the device.

All matmuls use fp16 operands (host pre-cast / pre-transposed) with fp32 PSUM
accumulation; softmax and LSTM gate math run in fp32. End-to-end output error
vs the fp32 reference is ~5e-4 (the fp16 input-rounding envelope).
"""
import sys
sys.path.insert(0, "/opt/trn_rl_repo")

import numpy as np
import concourse.mybir as mybir
import concourse.tile as tile
from concourse import bacc
from concourse.bass import ts, ds
from concourse.masks import make_identity
from concourse.tile_rust import add_dep_helper
from concourse.bass_utils import run_bass_kernel_spmd

dt = mybir.dt
AF = mybir.ActivationFunctionType

P = 128
S = 512        # source positions
ENC = 1024
DEC = 1024
EMB = 256
VOCAB = 32000
XDIM = ENC + EMB          # 1280
GATES = 4 * DEC           # 4096

VCHUNK = 500              # vocab tile (<=512 so one PSUM bank holds a chunk)
B_FULL = 128
N_CORES = 8


def build(R=16, n_cores=N_CORES):
    """Build the per-core Bass module. R = batch rows per core."""
    KO_E = ENC // P       # 8 k-chunks over ENC
    KO_X = XDIM // P      # 10
    ND = DEC // P         # 8 dec tiles
    NS = S // P           # 4 s chunks

    nc = bacc.Bacc("TRN2", target_bir_lowering=False, debug=False,
                   num_devices=n_cores)

    # ---- inputs ----
    encT = nc.dram_tensor("encT", (R, ENC, S), dt.float16, kind="ExternalInput").ap()
    encN = nc.dram_tensor("encN", (R, S, ENC), dt.float16, kind="ExternalInput").ap()
    dhT = nc.dram_tensor("dhT", (DEC, R), dt.float16, kind="ExternalInput").ap()
    w1 = nc.dram_tensor("w1", (ENC, DEC), dt.float16, kind="ExternalInput").ap()
    w2 = nc.dram_tensor("w2", (DEC, DEC), dt.float16, kind="ExternalInput").ap()
    b12 = nc.dram_tensor("b12", (DEC, 1), dt.float32, kind="ExternalInput").ap()
    wv = nc.dram_tensor("wv", (DEC, 1), dt.float16, kind="ExternalInput").ap()
    embT = nc.dram_tensor("embT", (EMB, R), dt.float16, kind="ExternalInput").ap()
    lstmk = nc.dram_tensor("lstmk", (XDIM, GATES), dt.float16, kind="ExternalInput").ap()
    lstmb = nc.dram_tensor("lstmb", (R, GATES), dt.float32, kind="ExternalInput").ap()
    vslice = VOCAB // n_cores                  # 4000 vocab per core
    nv = vslice // VCHUNK                      # 8 chunks
    fcw = nc.dram_tensor("fcw", (DEC, vslice), dt.float16, kind="ExternalInput").ap()

    # ---- outputs ----
    logits = nc.dram_tensor("logits", (B_FULL, vslice), dt.float32,
                            kind="ExternalOutput").ap()
    h_out = nc.dram_tensor("h_out", (R, DEC), dt.float32, kind="ExternalOutput").ap()
    c_out = nc.dram_tensor("c_out", (R, DEC), dt.float32, kind="ExternalOutput").ap()
    attn_out = nc.dram_tensor("attn_out", (R, S), dt.float32,
                              kind="ExternalOutput").ap()

    with tile.TileContext(nc) as tc:
        with (
            tc.tile_pool(name="consts", bufs=1) as cp,
            tc.tile_pool(name="psum", bufs=1, space="PSUM") as ps,
        ):
            # ---------- constants ----------
            w1_sb = cp.tile([P, KO_E, DEC], dt.float16, tag="w1")
            w1_r = w1.rearrange("(ko p) d -> p ko d", p=P)
            wv_sb = cp.tile([P, ND], dt.float16, tag="wv")
            nc.sync.dma_start(wv_sb[:], wv.rearrange("(ko p) one -> p (ko one)", p=P))
            b12_sb = cp.tile([P, ND], dt.float32, tag="b12")
            nc.sync.dma_start(b12_sb[:], b12.rearrange("(ko p) one -> p (ko one)", p=P))
            dhT_sb = cp.tile([P, ND, R], dt.float16, tag="dhT")
            nc.sync.dma_start(dhT_sb[:], dhT.rearrange("(ko p) b -> p ko b", p=P))

            id32 = cp.tile([P, P], dt.float32, tag="id32")
            make_identity(nc, id32[:])
            id16 = cp.tile([P, P], dt.float16, tag="id16")
            nc.vector.tensor_copy(id16[:], id32[:])

            xT_sb = cp.tile([P, KO_X, R], dt.float16, tag="xT")
            nc.sync.dma_start(xT_sb[:, ENC // P:, :],
                              embT.rearrange("(ko p) b -> p ko b", p=P))

            ctx_sb = cp.tile([R, ENC], dt.float32, tag="ctx")
            hpbT_sb = cp.tile([P, ND, R], dt.float32, tag="hpbT")

            # ---------- hidden_proj = dh @ W2 (transposed via PE), + (b1+b2) ----
            hp_sb = cp.tile([R, DEC], dt.float32, tag="hp")
            w2_r = w2.rearrange("(ko p) d -> p ko d", p=P)
            with tc.tile_pool(name="w2p", bufs=3) as w2p:
                for half in range(2):
                    ps_hp = ps.tile([R, 512], dt.float32, tag="E", bufs=4,
                                    name=f"hpm{half}")
                    for k in range(ND):
                        w2c = w2p.tile([P, 512], dt.float16, tag="w2c",
                                       name=f"w2c{half}_{k}")
                        nc.sync.dma_start(w2c[:], w2_r[:, k, ds(half * 512, 512)])
                        nc.tensor.matmul(ps_hp[:], dhT_sb[:, k, :], w2c[:],
                                         start=(k == 0), stop=(k == ND - 1))
                    nc.vector.tensor_copy(hp_sb[:, ds(half * 512, 512)], ps_hp[:])
                for d in range(ND):
                    ps_t = ps.tile([P, R], dt.float32, tag="tr", bufs=2,
                                   name=f"hpt{d}")
                    nc.tensor.transpose(ps_t[:], hp_sb[:, ts(d, P)], id32[:R, :R])
                    nc.vector.tensor_scalar(hpbT_sb[:, d, :], ps_t[:],
                                            b12_sb[:, d:d + 1], None,
                                            mybir.AluOpType.add)

            for _ko in range(KO_E):
                nc.sync.dma_start(w1_sb[:, _ko:_ko + 1, :], w1_r[:, _ko:_ko + 1, :])

            nat_dmas = []
            # ---------- per-row attention ----------
            main_pools = (
                tc.tile_pool(name="enc_nat", bufs=2),
                tc.tile_pool(name="enc_t", bufs=2),
                tc.tile_pool(name="tanh", bufs=3),
                tc.tile_pool(name="stream", bufs=2),
                tc.tile_pool(name="small", bufs=2),
            )
            pools = [p.__enter__() for p in main_pools]
            natp, etp, tp, strm, smp = pools
            for r in range(R):
                nat_t = natp.tile([P, NS, ENC], dt.float16, tag="nat", name=f"nat{r}")
                nat_dma = nc.sync.dma_start(
                    nat_t[:], encN[r].rearrange("(so p) e -> p so e", p=P))
                nat_dmas.append(nat_dma)
                et_t = etp.tile([P, KO_E, S], dt.float16, tag="et", name=f"et{r}")
                et_r = encT[r].rearrange("(ko p) s -> p ko s", p=P)
                for _ko in range(KO_E):
                    nc.sync.dma_start(et_t[:, _ko:_ko + 1, :], et_r[:, _ko:_ko + 1, :])

                # E = enc @ W1 laid out [dec, s]; two interleaved PSUM chains
                ps_sc = ps.tile([1, S], dt.float32, tag="row1", bufs=2, name=f"sc{r}")
                for d0 in range(0, ND, 2):
                    pes = [ps.tile([P, S], dt.float32, tag="E", bufs=4,
                                   name=f"E{r}_{d0 + j}") for j in range(2)]
                    for k in range(KO_E):
                        for j in range(2):
                            nc.tensor.matmul(pes[j][:], w1_sb[:, k, ts(d0 + j, P)],
                                             et_t[:, k, :],
                                             start=(k == 0), stop=(k == KO_E - 1))
                    for j in range(2):
                        d = d0 + j
                        t_t = tp.tile([P, S], dt.float16, tag="T", name=f"T{r}_{d}")
                        nc.scalar.activation(t_t[:], pes[j][:], AF.Tanh,
                                             bias=hpbT_sb[:, d, r:r + 1])
                        nc.tensor.matmul(ps_sc[:], wv_sb[:, d:d + 1], t_t[:],
                                         start=(d == 0), stop=(d == ND - 1))

                # softmax over s (bv is softmax-invariant and dropped)
                negmax = smp.tile([1, 1], dt.float32, tag="negmax", name=f"nm{r}")
                nc.vector.reduce_max(negmax[:], ps_sc[:], mybir.AxisListType.X,
                                     negate=True)
                sumexp = smp.tile([1, 1], dt.float32, tag="sumexp", name=f"se{r}")
                exp_t = smp.tile([1, S], dt.float32, tag="exp", bufs=1, name=f"ex{r}")
                nc.scalar.activation(exp_t[:], ps_sc[:], AF.Exp,
                                     bias=negmax[:, 0:1], accum_out=sumexp[:, 0:1])
                rec = smp.tile([1, 1], dt.float32, tag="rec", name=f"rc{r}")
                nc.vector.reciprocal(rec[:], sumexp[:])
                a32 = smp.tile([1, S], dt.float32, tag="a32", bufs=1, name=f"a32{r}")
                nc.vector.tensor_scalar_mul(a32[:], exp_t[:], rec[:, 0:1])
                nc.scalar.dma_start(attn_out[r:r + 1, :], a32[:])
                a16 = smp.tile([1, S], dt.float16, tag="a16", name=f"a16{r}")
                nc.vector.tensor_scalar_mul(a16[:], exp_t[:], rec[:, 0:1])

                # attn^T via PE transpose ([1,128] -> [128,1] per s-chunk)
                aT = smp.tile([P, NS], dt.float16, tag="aT", name=f"aT{r}")
                for sc in range(NS):
                    ps_at = ps.tile([P, 1], dt.float16, tag="tr", bufs=2,
                                    name=f"at{r}_{sc}")
                    nc.tensor.transpose(ps_at[:], a16[0:1, ts(sc, P)], id16[:1, :1])
                    nc.vector.tensor_copy(aT[:, sc:sc + 1], ps_at[:])

                # context^T chunks: attn^T.T @ enc_nat
                for eh in range(ENC // 512):
                    ps_cx = ps.tile([1, 512], dt.float32, tag="row1", bufs=2,
                                    name=f"cx{r}_{eh}")
                    for sc in range(NS):
                        nc.tensor.matmul(ps_cx[:], aT[:, sc:sc + 1],
                                         nat_t[:, sc, ds(eh * 512, 512)],
                                         start=(sc == 0), stop=(sc == NS - 1))
                    cxr = smp.tile([1, 512], dt.float32, tag="cxr", bufs=1,
                                   name=f"cxr{r}_{eh}")
                    nc.vector.tensor_copy(cxr[:], ps_cx[:])
                    nc.scalar.dma_start(ctx_sb[r:r + 1, ds(eh * 512, 512)], cxr[:])

            # ---------- x^T assembly: transpose context into xT ----------
            for d8 in range(ENC // P):
                ps_x = ps.tile([P, R], dt.float32, tag="tr", bufs=2, name=f"xt{d8}")
                nc.tensor.transpose(ps_x[:], ctx_sb[:, ts(d8, P)], id32[:R, :R])
                nc.vector.tensor_copy(xT_sb[:, d8, :], ps_x[:])

            # ---------- prefetch fc weights (own DMA queue, delayed) ----------
            fcw_r = fcw.rearrange("(ko p) v -> p ko v", p=P)
            fw_tiles = []
            for v in range(nv):
                fw_t = strm.tile([P, ND, VCHUNK], dt.float16, tag="fw", bufs=8,
                                 name=f"fw{v}")
                fw_dma = nc.gpsimd.dma_start(fw_t[:],
                                             fcw_r[:, :, ds(v * VCHUNK, VCHUNK)])
                add_dep_helper(fw_dma.ins, nat_dmas[13].ins, True, "delay fw")
                fw_tiles.append(fw_t)

            # ---------- LSTM: z = x @ lstm_k + b ----------
            # z chunks: i -> 0,1 ; g(cell) -> 4,5 ; o -> 6,7. The f gate is
            # multiplied by c0 = 0 in the reference, so its chunks 2,3 are skipped.
            Z_OFF = {0: 0, 1: 512, 4: 1024, 5: 1536, 6: 2048, 7: 2560}
            z_sb = cp.tile([R, 3 * DEC], dt.float32, tag="z")
            lstmk_r = lstmk.rearrange("(ko p) g -> p ko g", p=P)
            c_sb = cp.tile([R, DEC], dt.float32, tag="c")
            h_sb = cp.tile([R, DEC], dt.float32, tag="h")
            tmp_a = cp.tile([R, DEC], dt.float32, tag="tmp_a")
            tmp_b = cp.tile([R, DEC], dt.float32, tag="tmp_b")

            def z_chunk(g, ps_z_list):
                lk_t = strm.tile([P, KO_X, 512], dt.float16, tag="lk", bufs=3,
                                 name=f"lk{g}")
                lk_dma = nc.gpsimd.dma_start(lk_t[:], lstmk_r[:, :, ds(g * 512, 512)])
                add_dep_helper(lk_dma.ins, nat_dmas[10].ins, True, "delay lk")
                lb_t = strm.tile([R, 512], dt.float32, tag="fb", name=f"lb{g}")
                nc.sync.dma_start(lb_t[:], lstmb[:, ds(g * 512, 512)])
                ps_z = ps.tile([R, 512], dt.float32, tag="E", bufs=4, name=f"z{g}")
                ps_z_list.append((g, lk_t, lb_t, ps_z))

            for pair in ((0, 4), (1, 5), (6, 7)):
                zps = []
                for g in pair:
                    z_chunk(g, zps)
                for k in range(KO_X):
                    for g, lk_t, lb_t, ps_z in zps:
                        nc.tensor.matmul(ps_z[:], xT_sb[:, k, :], lk_t[:, k, :],
                                         start=(k == 0), stop=(k == KO_X - 1))
                for g, lk_t, lb_t, ps_z in zps:
                    nc.vector.tensor_tensor(z_sb[:, ds(Z_OFF[g], 512)], ps_z[:],
                                            lb_t[:], mybir.AluOpType.add)
                if pair[0] == 0:      # i0, g0 ready -> c half 0
                    nc.scalar.activation(tmp_a[:, 0:512], z_sb[:, 0:512], AF.Sigmoid)
                    nc.scalar.activation(tmp_b[:, 0:512], z_sb[:, 1024:1536], AF.Tanh)
                    nc.vector.tensor_mul(c_sb[:, 0:512], tmp_a[:, 0:512],
                                         tmp_b[:, 0:512])
                    nc.scalar.activation(tmp_b[:, 0:512], c_sb[:, 0:512], AF.Tanh)
                elif pair[0] == 1:    # i1, g1 -> c half 1
                    nc.scalar.activation(tmp_a[:, 512:1024], z_sb[:, 512:1024],
                                         AF.Sigmoid)
                    nc.scalar.activation(tmp_b[:, 512:1024], z_sb[:, 1536:2048],
                                         AF.Tanh)
                    nc.vector.tensor_mul(c_sb[:, 512:1024], tmp_a[:, 512:1024],
                                         tmp_b[:, 512:1024])
                    nc.scalar.activation(tmp_b[:, 512:1024], c_sb[:, 512:1024],
                                         AF.Tanh)
                else:                 # o ready -> h = sigmoid(o) * tanh(c)
                    nc.scalar.activation(tmp_a[:], z_sb[:, 2 * DEC:3 * DEC],
                                         AF.Sigmoid)
                    nc.vector.tensor_mul(h_sb[:], tmp_a[:], tmp_b[:])
            nc.gpsimd.dma_start(c_out, c_sb[:])
            nc.gpsimd.dma_start(h_out, h_sb[:])

            # ---------- all-gather h across cores (fp16) ----------
            h16_sb = cp.tile([R, DEC], dt.float16, tag="h16")
            nc.vector.tensor_copy(h16_sb[:], h_sb[:])
            with tc.tile_pool(name="dram", bufs=1, space="DRAM") as dram:
                h_shard = dram.tile([R, DEC], dt.float16, name="h_shard")
                h_full = dram.tile([B_FULL, DEC], dt.float16, name="h_full")
                nc.gpsimd.dma_start(h_shard[:], h16_sb[:])
                nc.gpsimd.collective_compute(
                    "AllGather", mybir.AluOpType.bypass,
                    replica_groups=[list(range(n_cores))],
                    ins=[h_shard.opt()], outs=[h_full.opt()])
                hf_sb = cp.tile([P, DEC], dt.float16, tag="hf")
                nc.gpsimd.dma_start(hf_sb[:], h_full[:])

            # h^T [dec, b_full] via PE transposes
            hT_sb = cp.tile([P, ND, B_FULL], dt.float16, tag="hT")
            for d8 in range(ND):
                ps_h = ps.tile([P, P], dt.float16, tag="tr", bufs=2, name=f"ht{d8}")
                nc.tensor.transpose(ps_h[:], hf_sb[:, ts(d8, P)], id16[:])
                nc.vector.tensor_copy(hT_sb[:, d8, :], ps_h[:])

            # ---------- fc on the vocab slice, full batch ----------
            for v0 in range(0, nv, 2):
                vps = []
                for v in (v0, v0 + 1):
                    ps_l = ps.tile([B_FULL, VCHUNK], dt.float32, tag="E", bufs=4,
                                   name=f"L{v}")
                    vps.append((v, fw_tiles[v], ps_l))
                for k in range(ND):
                    for v, fw_t, ps_l in vps:
                        nc.tensor.matmul(ps_l[:], hT_sb[:, k, :], fw_t[:, k, :],
                                         start=(k == 0), stop=(k == ND - 1))
                for v, fw_t, ps_l in vps:
                    lo_t = strm.tile([B_FULL, VCHUNK], dt.float32, tag="lo",
                                     name=f"lo{v}")
                    nc.vector.tensor_copy(lo_t[:], ps_l[:])
                    nc.scalar.dma_start(logits[:, ds(v * VCHUNK, VCHUNK)], lo_t[:])

            for p in reversed(main_pools):
                p.__exit__(None, None, None)

    nc.compile()
    return nc


# ---------------- host-side glue ----------------

def prep_inputs(inputs, decoder_hidden, encoder_output, emb, W1, b1, W2, b2,
                Wv, bv, lstm_k, lstm_rk, lstm_b, fc_W, fc_b, n_cores=N_CORES):
    """Full inputs -> (list of per-core in_maps, fc_b fp32 for the host-side add).

    lstm_rk multiplies h0 = 0 in the reference, so it is unused. bv shifts the
    softmax input uniformly and cancels; it is unused.
    """
    B = decoder_hidden.shape[0]
    R = B // n_cores
    f16 = np.float16
    idx = np.asarray(inputs)[:, 0].astype(np.int64)
    gathered = np.asarray(emb)[idx]                      # (B, EMB)

    enc16 = np.asarray(encoder_output).astype(f16)       # (B, S, ENC)
    encT16 = np.ascontiguousarray(enc16.transpose(0, 2, 1))  # (B, ENC, S)
    dh16 = np.asarray(decoder_hidden).astype(f16)
    b12v = (np.asarray(b1) + np.asarray(b2)).astype(np.float32)[:, None]
    w1_16 = np.asarray(W1).astype(f16)
    w2_16 = np.asarray(W2).astype(f16)
    wv16 = np.asarray(Wv).astype(f16)
    lstmk16 = np.asarray(lstm_k).astype(f16)
    lstmb32 = np.asarray(lstm_b).astype(np.float32)
    fcw16 = np.asarray(fc_W).astype(f16)
    fcb32 = np.asarray(fc_b).astype(np.float32)
    vslice = fcw16.shape[1] // n_cores

    in_maps = []
    for c in range(n_cores):
        rows = slice(c * R, (c + 1) * R)
        in_maps.append({
            "encT": np.ascontiguousarray(encT16[rows]),
            "encN": np.ascontiguousarray(enc16[rows]),
            "dhT": np.ascontiguousarray(dh16[rows].T),
            "w1": w1_16,
            "w2": w2_16,
            "b12": b12v,
            "wv": wv16,
            "embT": np.ascontiguousarray(gathered[rows].astype(f16).T),
            "lstmk": lstmk16,
            "lstmb": np.tile(lstmb32, (R, 1)),
            "fcw": np.ascontiguousarray(fcw16[:, c * vslice:(c + 1) * vslice]),
        })
    return in_maps, fcb32


def assemble_outputs(results, fcb32):
    """Per-core result dicts -> (logits, (h, c), attention_weights)."""
    logits = np.concatenate([r["logits"] for r in results], axis=1) + fcb32[None, :]
    logits = logits.astype(np.float32)
    h = np.concatenate([r["h_out"] for r in results], axis=0)
    c = np.concatenate([r["c_out"] for r in results], axis=0)
    attn = np.concatenate([r["attn_out"] for r in results], axis=0)[:, :, None]
    return logits, (h, c), attn


_cache = {}


def kernel(**inputs):
    """Full (unsharded) inputs -> full outputs, matching reference()'s structure:
    (logits, (h, c), attention_weights)."""
    if "nc" not in _cache:
        _cache["nc"] = build()
    nc = _cache["nc"]
    in_maps, fcb32 = prep_inputs(**inputs)
    res = run_bass_kernel_spmd(nc, in_maps, core_ids=list(range(N_CORES)))
    return assemble_outputs(res.results, fcb32)
